# revision 1
# baseline (speedup 1.0000x reference)
"""Trainium2 Bass kernel for nn_EnhancedDLinear (8-core SPMD, full I/O).

Mathematical reductions (verified against the jax reference, exact in fp32):

1. ``LayerNorm(1)`` in the reference normalizes over a size-1 axis, so
   ``(v - mean(v)) == 0`` exactly and its output is the constant ``ln_b``.
   Everything feeding it (detail decomposition, conv stack, adaptive
   softmax, the [N,S,S] self-attention) is dead code; ``detail_pred`` is a
   weight-only constant row, folded on the host.
2. The replicate-pad moving average (k=25) is a linear map ``mt = xc@Mm``;
   ``Mm`` folds into the first trend/seasonal MLP layers.
3. The channel-mean feeding the fusion MLP folds into its weights (1/96)
   and the constant detail contribution into its bias.

Sharding: the folded batch*channel axis (N = B*C) splits into 8 contiguous
blocks of C=96 = exactly one batch per core; each core runs one batch end
to end, zero collectives; tiny weights replicated.

Kernel structure (per core):
- Large matmuls (layer 1/2, softmax layer 2) run as float32r with moving
  dim >= 256 (1 cycle/row); data pre-rounded on host to the fp32r grid
  (sign+8exp+11mant). f32-consumed constants ship in separate f32 tensors
  (a DMA into an f32r tile rounds the payload).
- Layer 2 emits [tp | sp | rowsum(tp) | rowsum(sp)] in one PSUM tile via a
  block-diagonal weight matrix with appended column-sum columns.
- Per-partition biases ride the Relu activations; softmax-layer bias rides
  an augmented K=33 contraction (constant 1 in partition 32); rank-1
  output biases ride broadcast-DMAs + DVE adds.
- One row-contiguous DMA per tensor (DMA cost is per partition-row
  packet), spread across the three DMA-capable queues in need-order.
"""

import numpy as np

import concourse.bacc as bacc
import concourse.tile as tile
from concourse import mybir
from concourse.bass_utils import run_bass_kernel_spmd

B, S, C, P = 8, 336, 96, 96
HID = 168
MAIN_K = 25
N_CORES = 8
KC = 112          # K chunk (336 = 3*112)
NB = 256          # fp32r moving-dim block

M_TILES = [(0, 112), (112, 112), (224, 112)]

# wa [112, 1008] f32r: w1 K-chunks 0-2 (336 cols each)
# wb [112, 768]  f32r: w2 block-diag K-chunks (256 cols each)
WB_LEN = 768
# cf [112, 120] f32: colt (cols 0-2 b1 chunks, 4 b1f, 5 fp1b) | fnp
#    (fn1wT_t/96 | fn1wT_s/96 | fp1wT at cols 8:120)
# small [48, 1280] f32r rows/weights for f32r matmuls:
SM_FN2 = 0       # fn2wT rows 0:32 + fn2b row 32
SM_ONE = 768     # 1.0
SM_LEN = 1280
# sf [48, 608] f32: fp2wT (0:96) | [lt2b|ls2b|dp_row] row (96:384) |
#    fp2b row (384:480)
_CACHE = {}


def _round_fp32r(a):
    # fp32r keeps sign + 8 exp + 11 mantissa bits (low 12 bits zero);
    # round-to-nearest-even on the host so DMA'd bits are pre-rounded.
    u = np.ascontiguousarray(a, np.float32).view(np.uint32)
    low = u & np.uint32(0xFFF)
    base = u & ~np.uint32(0xFFF)
    up = (low > 0x800) | ((low == 0x800) & (((base >> 12) & 1) == 1))
    return (base + (up.astype(np.uint32) << 12)).view(np.float32)


def _mavg_matrix(s, k):
    # mt = xc @ Mm for the replicate-padded moving average
    p = (k - 1) // 2
    m = np.zeros((s, s), np.float64)
    for j in range(s):
        for d in range(-p, p + 1):
            i = min(max(j + d, 0), s - 1)
            m[i, j] += 1.0 / k
    return m.astype(np.float32)


def _build_module():
    f32 = mybir.dt.float32
    f32r = mybir.dt.float32r
    nc = bacc.Bacc("TRN2", target_bir_lowering=False, debug=False,
                   num_devices=N_CORES)

    xb = nc.dram_tensor("xb", [KC, 3 * NB], f32r, kind="ExternalInput")
    wa = nc.dram_tensor("wa", [KC, 3 * S], f32r, kind="ExternalInput")
    wb = nc.dram_tensor("wb", [KC, WB_LEN], f32r, kind="ExternalInput")
    small = nc.dram_tensor("small", [48, SM_LEN], f32r, kind="ExternalInput")
    cf = nc.dram_tensor("cf", [KC, 728], f32, kind="ExternalInput")
    y = nc.dram_tensor("y", [P, P], f32, kind="ExternalOutput")

    AF = mybir.ActivationFunctionType

    with tile.TileContext(nc) as tc:
        with (
            tc.tile_pool(name="wp", bufs=1) as wp,
            tc.tile_pool(name="hp", bufs=1) as hp,
            tc.tile_pool(name="pp", bufs=7, space="PSUM") as pp,
        ):
            xbs = wp.tile([KC, 3 * NB], f32r, tag="xbs")
            was = wp.tile([KC, 3 * S], f32r, tag="was")
            wbs = wp.tile([KC, WB_LEN], f32r, tag="wbs")
            small_s = wp.tile([48, SM_LEN], f32r, tag="small")
            cf_s = wp.tile([KC, 728], f32, tag="cf")

            nc.gpsimd.dma_start(out=was, in_=wa[:, :])
            nc.scalar.dma_start(out=xbs, in_=xb[:, :])
            nc.scalar.dma_start(out=cf_s, in_=cf[:, :])
            nc.sync.dma_start(out=wbs, in_=wb[:, :])
            nc.scalar.dma_start(out=small_s, in_=small[:, :])

            colt = cf_s[:, 0:8]
            fnp = cf_s[0:96, 8:120]
            fn2_s33 = small_s[0:33, SM_FN2:SM_FN2 + 288]
            one_r = small_s[0:1, SM_ONE:SM_ONE + 1]
            fp2_s = cf_s[0:48, 120:216]
            r3row = cf[0:1, 216:504]

            # [lt2b | ls2b | dp_row] and fp2b broadcast over 96 partitions
            r3b = hp.tile([96, 288], f32, tag="r3b")
            nc.gpsimd.dma_start(out=r3b, in_=r3row.broadcast_to((96, 288)))
            fp2bb = hp.tile([96, 96], f32, tag="fp2bb")
            nc.gpsimd.dma_start(out=fp2bb,
                                in_=cf[0:1, 504:600].broadcast_to((96, 96)))

            # ---- layer 1: h1T[u, c] = relu(W1.T @ xc_b.T + b1) ----
            h1c = [hp.tile([KC, 96], f32r, tag=f"h1c_{j}", name=f"h1c_{j}")
                   for j in range(3)]
            for i, (u0, us) in enumerate(M_TILES):
                ps = pp.tile([us, NB], f32, tag="ps")
                for j in range(3):
                    nc.tensor.matmul(
                        ps, was[:, S * j + u0:S * j + u0 + us],
                        xbs[:, NB * j:NB * (j + 1)],
                        start=(j == 0), stop=(j == 2))
                nc.scalar.activation(h1c[i], ps[:, 0:96], AF.Relu,
                                     bias=colt[0:us, i:i + 1])

            # ---- layer 2: [tp | sp | tps | sps] in one psum ----
            ps_l2 = pp.tile([96, NB], f32, tag="ps")
            for j in range(3):
                nc.tensor.matmul(ps_l2, h1c[j],
                                 wbs[:, NB * j:NB * (j + 1)],
                                 start=(j == 0), stop=(j == 2))

            ts2 = hp.tile([96, 2], f32, tag="ts2")
            nc.scalar.activation(ts2, ps_l2[:, 192:194], AF.Copy)
            # biased trend/seasonal blocks (off the softmax chain)
            at = hp.tile([96, 96], f32, tag="at")
            nc.vector.tensor_add(at, ps_l2[:, 0:96], r3b[:, 0:96])
            asl = hp.tile([96, 96], f32, tag="asl")
            nc.vector.tensor_add(asl, ps_l2[:, 96:192], r3b[:, 96:192])

            # ---- fusion softmax over 288 (row layout) ----
            ps_z1 = pp.tile([32, 1], f32, tag="ps")
            nc.tensor.matmul(ps_z1, fnp[:, 0:32], ts2[:, 0:1],
                             start=True, stop=False)
            nc.tensor.matmul(ps_z1, fnp[:, 32:64], ts2[:, 1:2],
                             start=False, stop=True)
            z1s = hp.tile([33, 1], f32r, tag="z1s")
            nc.sync.dma_start(out=z1s[32:33, 0:1],
                              in_=small[0:1, SM_ONE:SM_ONE + 1])
            nc.scalar.activation(z1s[0:32, 0:1], ps_z1, AF.Relu,
                                 bias=colt[0:32, 4:5])

            ps_z2 = pp.tile([1, 288], f32, tag="ps")
            nc.tensor.matmul(ps_z2, z1s, fn2_s33, start=True, stop=True)
            e_row = hp.tile([1, 288], f32, tag="e_row")
            den = hp.tile([1, 1], f32, tag="den")
            nc.scalar.activation(e_row, ps_z2, AF.Exp, accum_out=den)
            recip = hp.tile([1, 1], f32, tag="recip")
            nc.vector.reciprocal(recip, den)
            fw_row = hp.tile([1, 288], f32, tag="fw_row")
            nc.vector.tensor_scalar_mul(fw_row, e_row, recip[0:1, 0:1])

            # fw chunks -> per-partition columns via PE transpose
            fwcols = hp.tile([96, 3], f32, tag="fwcols")
            onef = one_r.bitcast(f32)
            ps_fw = pp.tile([96, 3], f32, tag="ps", name="ps_fw")
            for k in range(3):
                nc.tensor.matmul(ps_fw[:, k:k + 1],
                                 fw_row[0:1, 96 * k:96 * (k + 1)], onef,
                                 is_transpose=True, skip_group_check=True)
            nc.vector.tensor_copy(fwcols, ps_fw)

            # ---- G[c, p] = sum_k fw_k[c] * component_k[c, p] ----
            gt = hp.tile([96, 96], f32, tag="gt")
            nc.vector.tensor_scalar_mul(gt, at, fwcols[:, 0:1])
            gs = hp.tile([96, 96], f32, tag="gs")
            nc.vector.tensor_scalar_mul(gs, asl, fwcols[:, 1:2])
            gd = hp.tile([96, 96], f32, tag="gd")
            nc.vector.tensor_scalar_mul(gd, r3b[:, 192:288], fwcols[:, 2:3])
            ga = hp.tile([96, 96], f32, tag="ga")
            nc.vector.tensor_add(ga, gt, gs)
            g = hp.tile([96, 96], f32, tag="g")
            nc.vector.tensor_add(g, ga, gd)

            # ---- final projection (full fp32) ----
            ps_h = pp.tile([48, 96], f32, tag="ps")
            nc.tensor.matmul(ps_h, fnp[:, 64:112], g, start=True, stop=True)
            hs = hp.tile([48, 96], f32, tag="hs")
            nc.scalar.activation(hs, ps_h, AF.Relu, bias=colt[0:48, 5:6])

            ps_o = pp.tile([96, 96], f32, tag="ps")
            nc.tensor.matmul(ps_o, hs, fp2_s, start=True, stop=True)
            out_s = hp.tile([96, 96], f32, tag="out")
            nc.vector.tensor_add(out_s, ps_o, fp2bb)
            nc.sync.dma_start(out=y[:, :], in_=out_s)

    nc.compile()
    return nc


def _prep_weights(i):
    f = np.float32
    mm = _mavg_matrix(S, MAIN_K)
    w1 = np.empty((S, 2 * HID), f)
    w1[:, :HID] = mm @ i['lt1w'].T.astype(f)
    w1[:, HID:] = (np.eye(S, dtype=f) - mm) @ i['ls1w'].T.astype(f)
    wa = np.empty((KC, 3 * S), f)
    for j in range(3):
        wa[:, S * j:S * (j + 1)] = _round_fp32r(w1[KC * j:KC * (j + 1), :])

    # constant detail_pred row (LayerNorm(1) output == ln_b exactly)
    xf = np.full((S,), f(i['ln_b'][0]), f)
    dp_row = (np.maximum(xf @ i['op1w'].T + i['op1b'], 0)
              @ i['op2w'].T + i['op2b']).astype(f)
    dpm = dp_row.mean(dtype=np.float32)
    b1f = (i['fn1b']
           + dpm * i['fn1w'][:, 2 * C:].sum(1)
           + i['lt2b'].mean(dtype=np.float32) * i['fn1w'][:, 0:C].sum(1)
           + i['ls2b'].mean(dtype=np.float32) * i['fn1w'][:, C:2 * C].sum(1)
           ).astype(f)

    lt2wt = np.ascontiguousarray(i['lt2w'].T, f)
    ls2wt = np.ascontiguousarray(i['ls2w'].T, f)
    w2blk = np.zeros((S, NB), f)
    w2blk[0:HID, 0:96] = lt2wt
    w2blk[HID:, 96:192] = ls2wt
    w2blk[0:HID, 192] = lt2wt.sum(1)
    w2blk[HID:, 193] = ls2wt.sum(1)
    wb = np.zeros((KC, WB_LEN), f)
    for j in range(3):
        wb[:, NB * j:NB * (j + 1)] = _round_fp32r(w2blk[KC * j:KC * (j + 1)])

    b1 = np.concatenate([i['lt1b'], i['ls1b']]).astype(f)
    cf = np.zeros((KC, 728), f)
    for idx, (u0, us) in enumerate(M_TILES):
        cf[0:us, idx] = b1[u0:u0 + us]
    cf[0:32, 4] = b1f
    cf[0:48, 5] = i['fp1b']
    cf[0:96, 8:40] = i['fn1w'][:, 0:C].T / C
    cf[0:96, 40:72] = i['fn1w'][:, C:2 * C].T / C
    cf[0:96, 72:120] = i['fp1w'].T.astype(f)

    small = np.zeros((48, SM_LEN), f)
    small[0:32, SM_FN2:SM_FN2 + 288] = _round_fp32r(
        np.ascontiguousarray(i['fn2w'].T, f))
    small[32, SM_FN2:SM_FN2 + 288] = _round_fp32r(i['fn2b'].astype(f))
    small[0, SM_ONE] = 1.0

    cf[0:48, 120:216] = i['fp2w'].T.astype(f)
    cf[0, 216:312] = i['lt2b']
    cf[0, 312:408] = i['ls2b']
    cf[0, 408:504] = dp_row
    cf[0, 504:600] = i['fp2b']

    return dict(wa=wa, wb=wb, small=small, cf=cf)


def make_in_maps(inputs):
    shared = _prep_weights(inputs)
    x = np.asarray(inputs['x'], np.float32)
    in_maps = []
    for b in range(N_CORES):
        xbp = np.zeros((KC, 3 * NB), np.float32)
        for j in range(3):
            xbp[:, NB * j:NB * j + C] = x[b, KC * j:KC * (j + 1), :]
        in_maps.append(dict(shared, xb=_round_fp32r(xbp)))
    return in_maps


def kernel(**inputs):
    if "nc" not in _CACHE:
        _CACHE["nc"] = _build_module()
    res = run_bass_kernel_spmd(_CACHE["nc"], make_in_maps(inputs),
                               core_ids=list(range(N_CORES)))
    return np.stack([res.results[b]["y"] for b in range(N_CORES)], 0)

